# revision 41
# baseline (speedup 1.0000x reference)
"""Trainium2 Bass kernel for nn_Attention_47562467836169 (Bahdanau-style attention).

Reference math (S=4096, B=128, H=64):
    q = dec @ Wq_w.T + Wq_b                      # [B,1,H]
    k = enc @ Wk_w.T + Wk_b                      # [B,S,H]
    score = tanh(q + k) @ W_w.T + W_b            # [B,S,1]
    attn = softmax(score, axis=S)
    context = attn @ enc                         # [B,1,H]

Design (pair-major streaming; ~60us/exec vs 207us for the PE-transpose
baseline, measured via NTFF on 8 axon trn2 cores):
  * Pure data parallel over B: 16 batches/core as 8 PAIRS of 2 batches
    (2 x 64 h-channels = 128 partitions). The loop is PAIR-major: one
    pair's full 4096 s-rows per iteration, so the per-pair bias is
    CONSTANT per ACT instruction and bias+tanh+PSUM-eviction fuse into
    single [128,1024] ACT ops (the 352-cycle ACT overhead amortized; no
    separate DVE bias pass; ACT is the pacing engine at ~37us busy).
  * The encoder tensor is shipped in TWO host-prepared fp8-e4m3 layouts:
    h-partitioned (k-projection) and s-partitioned (context matmul) —
    zero on-device transposes, 8.5MB DMA/core. fp8 on both paths plus
    fp8 tanh output measures 1.15e-2 max rel err vs the f64 reference
    (gate 2e-2); bf16 encS would give 4.5e-3 at +4.25MB DMA.
  * The two streams ride different DMA paths so the 16 SDMA engines
    round-robin between queues (fills per-descriptor turnaround gaps):
    encT via sync HWDGE, encS via gpsimd SWDGE, both single_packet.
    Late-needed weights (v2/ones/ident) also go via gpsimd so the first
    encT stream starts ~2us earlier.
  * Matmul structure (FWL on all 128-col stationaries):
      - k:     stationary blockdiag(WkT,WkT) bf16, moving encT fp8
               [128,512] -> z in PSUM f32
      - v-dot: stationary tanh-chunk fp8 [128s,128] (FWL), moving
               v2 [128,2] -> scores come out s-PARTITIONED
      - ctx:   stationary enc-chunk fp8 [128s,128h2] (FWL), moving
               e-slice [128s,2], ONE PSUM accumulation group all kernel
      - den:   ones^T @ e per pair, DVE-reduced into den16 incrementally
  * softmax needs no max-subtraction (|score| <= sum|W_w| ~ 8); W_b is
    softmax-invariant and dropped; normalization happens once in the
    tail (transpose-via-PE + DVE reciprocal + per-partition scale).
  * exp(p) / den(p) / ctx(p) are deferred until after pair p+1's tanh is
    enqueued so ACT never stalls on pair p's v-dots (ACT stream runs
    ~92% packed).
  * PSUM: 6 banks z (3 x [128,1024] f32 waves -> PE runs 3 waves ahead,
    HAM stays warm), 1 bank scores+den (one-shot writes, 4-slot
    rotation), 1 bank ctx+tail scratch (also hosts the bias2 preamble).
  * Startup: pair-0's two streams are the FIRST instructions on their DMA
    queues (the tiny weights queue behind them, preamble runs off the cd
    bank) — worth ~10us of ramp vs issuing the singles first. A few tiny
    PE warmup matmuls bridge the first DMA wait for the HAM clock gate.
"""

import numpy as np

S, B, H = 4096, 128, 64
NCORES = 8
BC = B // NCORES          # batches per core = 16
PAIRS = BC // 2           # 8
NC_ = S // 128            # 128-row chunks per pair = 32
WAVE = 1024               # s rows per z wave
NWAVE = S // WAVE         # 4

_CACHE = {}


def _build_nc(nblk=None, reps=1):
    import concourse.bacc as bacc
    import concourse.tile as tile
    from concourse import mybir

    f32 = mybir.dt.float32
    bf16 = mybir.dt.bfloat16
    fp8 = mybir.dt.float8e4
    Act = mybir.ActivationFunctionType

    nc = bacc.Bacc(None, target_bir_lowering=False)
    encT_d = nc.dram_tensor("enct", [PAIRS, 128, S], fp8, kind="ExternalInput")
    encS_d = nc.dram_tensor("encs", [PAIRS, 128, NC_, 128], fp8, kind="ExternalInput")
    decT_d = nc.dram_tensor("dect", [H + 2, BC], f32, kind="ExternalInput")
    wqT_d = nc.dram_tensor("wqt", [H + 2, H], f32, kind="ExternalInput")
    wk2_d = nc.dram_tensor("wk2", [128, 128], bf16, kind="ExternalInput")
    v2_d = nc.dram_tensor("v2", [128, 2], bf16, kind="ExternalInput")
    id_d = nc.dram_tensor("ident", [128, 128], f32, kind="ExternalInput")
    ones_d = nc.dram_tensor("ones1", [128, 1], bf16, kind="ExternalInput")
    out_d = nc.dram_tensor("outp", [BC, 128], f32, kind="ExternalOutput")

    with tile.TileContext(nc) as tc:
        with tc.tile_pool(name="singles", bufs=1) as singles:
            wk2_sb = singles.tile([128, 128], bf16)
            v2_sb = singles.tile([128, 2], bf16)
            id_sb = singles.tile([128, 128], f32)
            decT_sb = singles.tile([H + 2, BC], f32)
            wqT_sb = singles.tile([H + 2, H], f32)
            ones_sb = singles.tile([128, 1], bf16)
            bias2_sb = singles.tile([128, PAIRS], f32)
            one32_sb = singles.tile([1, 1], f32)
            warm_sb = singles.tile([1, 2], f32)
            den16_sb = singles.tile([1, BC], f32)
            rden_sb = singles.tile([BC, 1], f32)
            ctx_sb = singles.tile([128, BC], f32)
            outp_sb = singles.tile([BC, 128], f32)
            nc.vector.memset(one32_sb[:], 1.0)
            nc.vector.memset(warm_sb[:], 0.0)

            # ---- main streaming loop (pair-major)
            with (
                tc.tile_pool(name="etT", bufs=4) as etTp,
                tc.tile_pool(name="etS", bufs=4) as etSp,
                tc.tile_pool(name="thp", bufs=3) as thp,
                tc.tile_pool(name="epool", bufs=3) as epool,
                tc.tile_pool(name="psZ", bufs=3, space="PSUM") as psZ,
                tc.tile_pool(name="psS", bufs=1, space="PSUM") as psS,
                tc.tile_pool(name="psC", bufs=1, space="PSUM") as psC,
            ):
                # one bank, one-shot writes only: scores(p) at 64*(p%4),
                # den(p) at 256+64*(p%4); den is DVE-reduced into den16
                # before its slot is reused 4 pairs later.
                sc_ps = psS.tile([128, 512], f32, tag="sc")
                # one bank: cols 0:16 ctx, 80:81 rden^T, 128:256 ctx^T
                # (128:136 also hosts the one-shot bias2 preamble early on)
                cd_ps = psC.tile([128, 256], f32, tag="cd")

                # pair-0 streams issue FIRST on both queues so the k-matmul
                # pipeline is fed as early as possible; the tiny weights
                # queue up right behind them.
                pre0_etT = etTp.tile([128, S], fp8, tag="etT")
                nc.sync.dma_start(pre0_etT[:], encT_d[0], single_packet=True)
                nc.gpsimd.dma_start(wk2_sb[:], wk2_d[:])
                pre0_etS = etSp.tile([128, NC_, 128], fp8, tag="etS")
                nc.gpsimd.dma_start(pre0_etS[:], encS_d[0], single_packet=True)
                nc.sync.dma_start(decT_sb[:], decT_d[:])
                nc.sync.dma_start(wqT_sb[:], wqT_d[:])
                nc.gpsimd.dma_start(v2_sb[:], v2_d[:])
                nc.gpsimd.dma_start(ones_sb[:], ones_d[:])
                nc.gpsimd.dma_start(id_sb[:], id_d[:])

                # Pull the exp_and_others ACT table (tanh+exp) off the
                # critical path, overlapped with the first pair's DMA.
                nc.scalar.activation(warm_sb[:, 1:2], warm_sb[:, 0:1], Act.Tanh)

                # preamble: bias2[64j+h', p] = (dec[2p+j] @ WqT + Wq_b + Wk_b)[h']
                nc.tensor.matmul(cd_ps[0:H, 128:136], wqT_sb[:],
                                 decT_sb[:, 0:PAIRS], start=True, stop=True,
                                 skip_group_check=True)
                nc.tensor.matmul(cd_ps[H:2 * H, 128:136], wqT_sb[:],
                                 decT_sb[:, PAIRS:BC], start=True, stop=True,
                                 tile_position=(0, 64), skip_group_check=True)
                nc.vector.tensor_copy(bias2_sb[:], cd_ps[:, 128:136])

                for rep in range(reps):
                    # softmax+ctx for pair p are deferred until after pair
                    # p+1's tanh is enqueued, so the ACT engine never stalls
                    # waiting for pair p's v-dots.
                    def soft_ctx(p, etS):
                        c0 = 64 * (p % 4)
                        e_sb = epool.tile([128, 64], bf16, tag="e")
                        nc.scalar.activation(e_sb[:], sc_ps[:, c0:c0 + 64], Act.Exp)
                        dc = 256 + 64 * (p % 4)
                        nc.tensor.matmul(sc_ps[0:1, dc:dc + 64], ones_sb[:],
                                         e_sb[:], start=True, stop=True,
                                         skip_group_check=True)
                        den_r = sc_ps[0:1, dc:dc + 64].rearrange(
                            "o (c j) -> o j c", j=2)
                        nc.vector.tensor_reduce(
                            den16_sb[0:1, 2 * p:2 * p + 2], den_r,
                            axis=mybir.AxisListType.X, op=mybir.AluOpType.add)
                        for c in range(NC_):
                            nc.tensor.matmul(
                                cd_ps[:, 2 * p:2 * p + 2],
                                etS[:, c, :],
                                e_sb[:, 2 * c:2 * c + 2],
                                start=(p == 0 and c == 0),
                                stop=(p == PAIRS - 1 and c == NC_ - 1),
                                skip_group_check=True)

                    prev = None
                    for p in range(PAIRS):
                        if rep == 0 and p == 0:
                            etT, etS = pre0_etT, pre0_etS
                        else:
                            etT = etTp.tile([128, S], fp8, tag="etT")
                            nc.sync.dma_start(etT[:], encT_d[p], single_packet=True)
                            etS = etSp.tile([128, NC_, 128], fp8, tag="etS")
                            nc.gpsimd.dma_start(etS[:], encS_d[p], single_packet=True)
                        th = thp.tile([128, S], fp8, tag="th")
                        for w in range(NWAVE):
                            zw = psZ.tile([128, WAVE], f32, tag="z")
                            for g in range(2):
                                o = WAVE * w + 512 * g
                                nc.tensor.matmul(zw[:, 512 * g:512 * (g + 1)],
                                                 wk2_sb[:], etT[:, o:o + 512],
                                                 start=True, stop=True)
                            nc.scalar.activation(th[:, WAVE * w:WAVE * (w + 1)],
                                                 zw[:], Act.Tanh,
                                                 bias=bias2_sb[:, p:p + 1])
                        c0 = 64 * (p % 4)
                        for c in range(NC_):
                            nc.tensor.matmul(
                                sc_ps[:, c0 + 2 * c:c0 + 2 * c + 2],
                                th[:, 128 * c:128 * (c + 1)],
                                v2_sb[:], start=True, stop=True)
                        if prev is not None:
                            soft_ctx(*prev)
                        prev = (p, etS)
                    soft_ctx(*prev)

                    # ---- tail: normalize and emit
                    nc.tensor.matmul(cd_ps[0:BC, 80:81], den16_sb[0:1, :],
                                     one32_sb[0:1, 0:1], is_transpose=True,
                                     start=True, stop=True, skip_group_check=True)
                    nc.vector.reciprocal(rden_sb[:], cd_ps[0:BC, 80:81])
                    nc.vector.tensor_copy(ctx_sb[:], cd_ps[:, 0:BC])
                    nc.tensor.matmul(cd_ps[0:BC, 128:256], ctx_sb[:], id_sb[:],
                                     is_transpose=True,
                                     start=True, stop=True, skip_group_check=True)
                    nc.vector.tensor_scalar_mul(outp_sb[:], cd_ps[0:BC, 128:256],
                                                rden_sb[:, 0:1])
                    nc.sync.dma_start(out_d[:], outp_sb[:])
    nc.compile()
    return nc


def get_nc(nblk=None, reps=1):
    key = (nblk, reps)
    if key not in _CACHE:
        _CACHE[key] = _build_nc(nblk, reps)
    return _CACHE[key]


def host_prep(enc, dec, wq_w, wq_b, wk_w, wk_b, w_w, nblk=None):
    """Build the 8 per-core input maps. enc [S,B,H] f32, dec [B,H]."""
    import ml_dtypes

    bf16 = ml_dtypes.bfloat16
    fp8 = ml_dtypes.float8_e4m3
    wk2 = np.zeros((128, 128), np.float32)
    wk2[0:H, 0:H] = wk_w.T
    wk2[H:2 * H, H:2 * H] = wk_w.T
    wk2 = wk2.astype(bf16)
    v2 = np.zeros((128, 2), np.float32)
    v2[0:H, 0] = w_w[0]
    v2[H:2 * H, 1] = w_w[0]
    v2 = v2.astype(bf16)
    ident = np.eye(128, dtype=np.float32)
    wqT = np.zeros((H + 2, H), np.float32)
    wqT[0:H] = wq_w.T
    wqT[H] = wq_b
    wqT[H + 1] = wk_b
    ones1 = np.ones((128, 1), np.float32).astype(bf16)
    in_maps = []
    for c in range(NCORES):
        e = enc[:, BC * c:BC * (c + 1), :]            # [S, 16, 64] f32
        # h-partitioned: encT[p, 64j+h, s] = e[s, 2p+j, h]
        et = e.reshape(S, PAIRS, 2, H)                 # [s, p, j, h]
        encT = np.ascontiguousarray(
            et.transpose(1, 2, 3, 0).reshape(PAIRS, 128, S)
        ).astype(fp8)
        # s-partitioned: encS[p, i, c, 64j+h] = e[128c+i, 2p+j, h]
        es = e.reshape(NC_, 128, PAIRS, 2 * H)         # [c, i, p, jh]
        encS = np.ascontiguousarray(
            es.transpose(2, 1, 0, 3).reshape(PAIRS, 128, NC_, 128)
        ).astype(fp8)
        d = dec[BC * c:BC * (c + 1)]                  # [16, 64]
        decT = np.ones((H + 2, BC), np.float32)
        decT[0:H, 0:PAIRS] = d[0::2].T
        decT[0:H, PAIRS:BC] = d[1::2].T
        in_maps.append({
            "enct": encT, "encs": encS,
            "dect": decT, "wqt": wqT, "wk2": wk2, "v2": v2, "ident": ident,
            "ones1": ones1,
        })
    return in_maps


def assemble_output(results):
    """results: list of 8 dicts with 'outp' [16,128] -> full [1,B,H]."""
    out = np.zeros((1, B, H), np.float32)
    for c in range(NCORES):
        o = results[c]["outp"]                        # [16, 128]
        for r in range(BC):
            j = r % 2
            out[0, BC * c + r, :] = o[r, H * j:H * (j + 1)]
    return out


def kernel(encoder_outputs, decoder_hidden, Wq_w, Wq_b, Wk_w, Wk_b, W_w, W_b,
           **kwargs):
    from concourse.bass_utils import run_bass_kernel_spmd

    enc = np.asarray(encoder_outputs, np.float32)
    dec = np.asarray(decoder_hidden, np.float32)[0]
    in_maps = host_prep(enc, dec,
                        np.asarray(Wq_w, np.float32), np.asarray(Wq_b, np.float32),
                        np.asarray(Wk_w, np.float32), np.asarray(Wk_b, np.float32),
                        np.asarray(W_w, np.float32))
    nc = get_nc()
    res = run_bass_kernel_spmd(nc, in_maps, core_ids=list(range(NCORES)))
    return assemble_output(res.results)


# revision 42
# speedup vs baseline: 1.1502x; 1.1502x over previous
"""Trainium2 Bass kernel for nn_Attention_47562467836169 (Bahdanau-style attention).

Reference math (S=4096, B=128, H=64):
    q = dec @ Wq_w.T + Wq_b                      # [B,1,H]
    k = enc @ Wk_w.T + Wk_b                      # [B,S,H]
    score = tanh(q + k) @ W_w.T + W_b            # [B,S,1]
    attn = softmax(score, axis=S)
    context = attn @ enc                         # [B,1,H]

Design (pair-major streaming; ~60us/exec vs 207us for the PE-transpose
baseline, measured via NTFF on 8 axon trn2 cores):
  * Pure data parallel over B: 16 batches/core as 8 PAIRS of 2 batches
    (2 x 64 h-channels = 128 partitions). The loop is PAIR-major: one
    pair's full 4096 s-rows per iteration, so the per-pair bias is
    CONSTANT per ACT instruction and bias+tanh+PSUM-eviction fuse into
    single [128,1024] ACT ops (the 352-cycle ACT overhead amortized; no
    separate DVE bias pass; ACT is the pacing engine at ~37us busy).
  * The encoder tensor is shipped in TWO host-prepared fp8-e4m3 layouts:
    h-partitioned (k-projection) and s-partitioned (context matmul) —
    zero on-device transposes, 8.5MB DMA/core. fp8 on both paths plus
    fp8 tanh output measures 1.15e-2 max rel err vs the f64 reference
    (gate 2e-2); bf16 encS would give 4.5e-3 at +4.25MB DMA.
  * The two streams ride different DMA paths so the 16 SDMA engines
    round-robin between queues (fills per-descriptor turnaround gaps):
    encT via sync HWDGE, encS via gpsimd SWDGE, both single_packet.
    Late-needed weights (v2/ones/ident) also go via gpsimd so the first
    encT stream starts ~2us earlier.
  * Matmul structure (FWL on all 128-col stationaries):
      - k:     stationary blockdiag(WkT,WkT) bf16, moving encT fp8
               [128,512] -> z in PSUM f32
      - v-dot: stationary tanh-chunk fp8 [128s,128] (FWL), moving
               v2 [128,2] -> scores come out s-PARTITIONED
      - ctx:   stationary enc-chunk fp8 [128s,128h2] (FWL), moving
               e-slice [128s,2], ONE PSUM accumulation group all kernel
      - den:   ones^T @ e per pair, DVE-reduced into den16 incrementally
  * softmax needs no max-subtraction (|score| <= sum|W_w| ~ 8); W_b is
    softmax-invariant and dropped; normalization happens once in the
    tail (transpose-via-PE + DVE reciprocal + per-partition scale).
  * exp(p) / den(p) / ctx(p) are deferred until after pair p+1's tanh is
    enqueued so ACT never stalls on pair p's v-dots (ACT stream runs
    ~92% packed).
  * PSUM: 6 banks z (3 x [128,1024] f32 waves -> PE runs 3 waves ahead,
    HAM stays warm), 1 bank scores+den (one-shot writes, 4-slot
    rotation), 1 bank ctx+tail scratch (also hosts the bias2 preamble).
  * Startup: pair-0's two streams are the FIRST instructions on their DMA
    queues (the tiny weights queue behind them, preamble runs off the cd
    bank, bias copy on the DVE) — worth ~10us of ramp vs singles-first.
"""

import numpy as np

S, B, H = 4096, 128, 64
NCORES = 8
BC = B // NCORES          # batches per core = 16
PAIRS = BC // 2           # 8
NC_ = S // 128            # 128-row chunks per pair = 32
WAVE = 1024               # s rows per z wave
NWAVE = S // WAVE         # 4

_CACHE = {}


def _build_nc(nblk=None, reps=1):
    import concourse.bacc as bacc
    import concourse.tile as tile
    from concourse import mybir

    f32 = mybir.dt.float32
    bf16 = mybir.dt.bfloat16
    fp8 = mybir.dt.float8e4
    Act = mybir.ActivationFunctionType

    nc = bacc.Bacc(None, target_bir_lowering=False)
    encT_d = nc.dram_tensor("enct", [PAIRS, 128, S], fp8, kind="ExternalInput")
    encS_d = nc.dram_tensor("encs", [PAIRS, 128, NC_, 128], fp8, kind="ExternalInput")
    decT_d = nc.dram_tensor("dect", [H + 2, BC], f32, kind="ExternalInput")
    wqT_d = nc.dram_tensor("wqt", [H + 2, H], f32, kind="ExternalInput")
    wk2_d = nc.dram_tensor("wk2", [128, 128], bf16, kind="ExternalInput")
    v2_d = nc.dram_tensor("v2", [128, 2], bf16, kind="ExternalInput")
    id_d = nc.dram_tensor("ident", [128, 128], f32, kind="ExternalInput")
    ones_d = nc.dram_tensor("ones1", [128, 1], bf16, kind="ExternalInput")
    out_d = nc.dram_tensor("outp", [BC, 128], f32, kind="ExternalOutput")

    with tile.TileContext(nc) as tc:
        with tc.tile_pool(name="singles", bufs=1) as singles:
            wk2_sb = singles.tile([128, 128], bf16)
            v2_sb = singles.tile([128, 2], bf16)
            id_sb = singles.tile([128, 128], f32)
            decT_sb = singles.tile([H + 2, BC], f32)
            wqT_sb = singles.tile([H + 2, H], f32)
            ones_sb = singles.tile([128, 1], bf16)
            bias2_sb = singles.tile([128, PAIRS], f32)
            one32_sb = singles.tile([1, 1], f32)
            warm_sb = singles.tile([1, 2], f32)
            den16_sb = singles.tile([1, BC], f32)
            rden_sb = singles.tile([BC, 1], f32)
            ctx_sb = singles.tile([128, BC], f32)
            outp_sb = singles.tile([BC, 128], f32)
            nc.vector.memset(one32_sb[:], 1.0)
            nc.vector.memset(warm_sb[:], 0.0)

            # ---- main streaming loop (pair-major)
            with (
                tc.tile_pool(name="etT", bufs=4) as etTp,
                tc.tile_pool(name="etS", bufs=4) as etSp,
                tc.tile_pool(name="thp", bufs=3) as thp,
                tc.tile_pool(name="epool", bufs=3) as epool,
                tc.tile_pool(name="psZ", bufs=3, space="PSUM") as psZ,
                tc.tile_pool(name="psS", bufs=1, space="PSUM") as psS,
                tc.tile_pool(name="psC", bufs=1, space="PSUM") as psC,
            ):
                # one bank, one-shot writes only: scores(p) at 64*(p%4),
                # den(p) at 256+64*(p%4); den is DVE-reduced into den16
                # before its slot is reused 4 pairs later.
                sc_ps = psS.tile([128, 512], f32, tag="sc")
                # one bank: cols 0:16 ctx, 80:81 rden^T, 128:256 ctx^T
                # (128:136 also hosts the one-shot bias2 preamble early on)
                cd_ps = psC.tile([128, 256], f32, tag="cd")

                # pair-0 streams issue FIRST on both queues so the k-matmul
                # pipeline is fed as early as possible; the tiny weights
                # queue up right behind them.
                pre0_etT = etTp.tile([128, S], fp8, tag="etT")
                nc.sync.dma_start(pre0_etT[:], encT_d[0], single_packet=True)
                pre0_etS = etSp.tile([128, NC_, 128], fp8, tag="etS")
                nc.gpsimd.dma_start(pre0_etS[:], encS_d[0], single_packet=True)
                nc.sync.dma_start(wk2_sb[:], wk2_d[:])
                nc.sync.dma_start(decT_sb[:], decT_d[:])
                nc.sync.dma_start(wqT_sb[:], wqT_d[:])
                nc.gpsimd.dma_start(v2_sb[:], v2_d[:])
                nc.gpsimd.dma_start(ones_sb[:], ones_d[:])
                nc.gpsimd.dma_start(id_sb[:], id_d[:])

                # Pull the exp_and_others ACT table (tanh+exp) off the
                # critical path, overlapped with the first pair's DMA.
                nc.scalar.activation(warm_sb[:, 1:2], warm_sb[:, 0:1], Act.Tanh)

                # preamble: bias2[64j+h', p] = (dec[2p+j] @ WqT + Wq_b + Wk_b)[h']
                nc.tensor.matmul(cd_ps[0:H, 128:136], wqT_sb[:],
                                 decT_sb[:, 0:PAIRS], start=True, stop=True,
                                 skip_group_check=True)
                nc.tensor.matmul(cd_ps[H:2 * H, 128:136], wqT_sb[:],
                                 decT_sb[:, PAIRS:BC], start=True, stop=True,
                                 tile_position=(0, 64), skip_group_check=True)
                nc.vector.tensor_copy(bias2_sb[:], cd_ps[:, 128:136])

                for rep in range(reps):
                    # softmax+ctx for pair p are deferred until after pair
                    # p+1's tanh is enqueued, so the ACT engine never stalls
                    # waiting for pair p's v-dots.
                    def soft_ctx(p, etS):
                        c0 = 64 * (p % 4)
                        e_sb = epool.tile([128, 64], bf16, tag="e")
                        nc.scalar.activation(e_sb[:], sc_ps[:, c0:c0 + 64], Act.Exp)
                        dc = 256 + 64 * (p % 4)
                        nc.tensor.matmul(sc_ps[0:1, dc:dc + 64], ones_sb[:],
                                         e_sb[:], start=True, stop=True,
                                         skip_group_check=True)
                        den_r = sc_ps[0:1, dc:dc + 64].rearrange(
                            "o (c j) -> o j c", j=2)
                        nc.vector.tensor_reduce(
                            den16_sb[0:1, 2 * p:2 * p + 2], den_r,
                            axis=mybir.AxisListType.X, op=mybir.AluOpType.add)
                        for c in range(NC_):
                            nc.tensor.matmul(
                                cd_ps[:, 2 * p:2 * p + 2],
                                etS[:, c, :],
                                e_sb[:, 2 * c:2 * c + 2],
                                start=(p == 0 and c == 0),
                                stop=(p == PAIRS - 1 and c == NC_ - 1),
                                skip_group_check=True)

                    prev = None
                    for p in range(PAIRS):
                        if rep == 0 and p == 0:
                            etT, etS = pre0_etT, pre0_etS
                        else:
                            etT = etTp.tile([128, S], fp8, tag="etT")
                            nc.sync.dma_start(etT[:], encT_d[p], single_packet=True)
                            etS = etSp.tile([128, NC_, 128], fp8, tag="etS")
                            nc.gpsimd.dma_start(etS[:], encS_d[p], single_packet=True)
                        th = thp.tile([128, S], fp8, tag="th")
                        for w in range(NWAVE):
                            zw = psZ.tile([128, WAVE], f32, tag="z")
                            for g in range(2):
                                o = WAVE * w + 512 * g
                                nc.tensor.matmul(zw[:, 512 * g:512 * (g + 1)],
                                                 wk2_sb[:], etT[:, o:o + 512],
                                                 start=True, stop=True)
                            nc.scalar.activation(th[:, WAVE * w:WAVE * (w + 1)],
                                                 zw[:], Act.Tanh,
                                                 bias=bias2_sb[:, p:p + 1])
                        c0 = 64 * (p % 4)
                        for c in range(NC_):
                            nc.tensor.matmul(
                                sc_ps[:, c0 + 2 * c:c0 + 2 * c + 2],
                                th[:, 128 * c:128 * (c + 1)],
                                v2_sb[:], start=True, stop=True)
                        if prev is not None:
                            soft_ctx(*prev)
                        prev = (p, etS)
                    soft_ctx(*prev)

                    # ---- tail: normalize and emit
                    nc.tensor.matmul(cd_ps[0:BC, 80:81], den16_sb[0:1, :],
                                     one32_sb[0:1, 0:1], is_transpose=True,
                                     start=True, stop=True, skip_group_check=True)
                    nc.vector.reciprocal(rden_sb[:], cd_ps[0:BC, 80:81])
                    nc.vector.tensor_copy(ctx_sb[:], cd_ps[:, 0:BC])
                    nc.tensor.matmul(cd_ps[0:BC, 128:256], ctx_sb[:], id_sb[:],
                                     is_transpose=True,
                                     start=True, stop=True, skip_group_check=True)
                    nc.vector.tensor_scalar_mul(outp_sb[:], cd_ps[0:BC, 128:256],
                                                rden_sb[:, 0:1])
                    nc.sync.dma_start(out_d[:], outp_sb[:])
    nc.compile()
    return nc


def get_nc(nblk=None, reps=1):
    key = (nblk, reps)
    if key not in _CACHE:
        _CACHE[key] = _build_nc(nblk, reps)
    return _CACHE[key]


def host_prep(enc, dec, wq_w, wq_b, wk_w, wk_b, w_w, nblk=None):
    """Build the 8 per-core input maps. enc [S,B,H] f32, dec [B,H]."""
    import ml_dtypes

    bf16 = ml_dtypes.bfloat16
    fp8 = ml_dtypes.float8_e4m3
    wk2 = np.zeros((128, 128), np.float32)
    wk2[0:H, 0:H] = wk_w.T
    wk2[H:2 * H, H:2 * H] = wk_w.T
    wk2 = wk2.astype(bf16)
    v2 = np.zeros((128, 2), np.float32)
    v2[0:H, 0] = w_w[0]
    v2[H:2 * H, 1] = w_w[0]
    v2 = v2.astype(bf16)
    ident = np.eye(128, dtype=np.float32)
    wqT = np.zeros((H + 2, H), np.float32)
    wqT[0:H] = wq_w.T
    wqT[H] = wq_b
    wqT[H + 1] = wk_b
    ones1 = np.ones((128, 1), np.float32).astype(bf16)
    in_maps = []
    for c in range(NCORES):
        e = enc[:, BC * c:BC * (c + 1), :]            # [S, 16, 64] f32
        # h-partitioned: encT[p, 64j+h, s] = e[s, 2p+j, h]
        et = e.reshape(S, PAIRS, 2, H)                 # [s, p, j, h]
        encT = np.ascontiguousarray(
            et.transpose(1, 2, 3, 0).reshape(PAIRS, 128, S)
        ).astype(fp8)
        # s-partitioned: encS[p, i, c, 64j+h] = e[128c+i, 2p+j, h]
        es = e.reshape(NC_, 128, PAIRS, 2 * H)         # [c, i, p, jh]
        encS = np.ascontiguousarray(
            es.transpose(2, 1, 0, 3).reshape(PAIRS, 128, NC_, 128)
        ).astype(fp8)
        d = dec[BC * c:BC * (c + 1)]                  # [16, 64]
        decT = np.ones((H + 2, BC), np.float32)
        decT[0:H, 0:PAIRS] = d[0::2].T
        decT[0:H, PAIRS:BC] = d[1::2].T
        in_maps.append({
            "enct": encT, "encs": encS,
            "dect": decT, "wqt": wqT, "wk2": wk2, "v2": v2, "ident": ident,
            "ones1": ones1,
        })
    return in_maps


def assemble_output(results):
    """results: list of 8 dicts with 'outp' [16,128] -> full [1,B,H]."""
    out = np.zeros((1, B, H), np.float32)
    for c in range(NCORES):
        o = results[c]["outp"]                        # [16, 128]
        for r in range(BC):
            j = r % 2
            out[0, BC * c + r, :] = o[r, H * j:H * (j + 1)]
    return out


def kernel(encoder_outputs, decoder_hidden, Wq_w, Wq_b, Wk_w, Wk_b, W_w, W_b,
           **kwargs):
    from concourse.bass_utils import run_bass_kernel_spmd

    enc = np.asarray(encoder_outputs, np.float32)
    dec = np.asarray(decoder_hidden, np.float32)[0]
    in_maps = host_prep(enc, dec,
                        np.asarray(Wq_w, np.float32), np.asarray(Wq_b, np.float32),
                        np.asarray(Wk_w, np.float32), np.asarray(Wk_b, np.float32),
                        np.asarray(W_w, np.float32))
    nc = get_nc()
    res = run_bass_kernel_spmd(nc, in_maps, core_ids=list(range(NCORES)))
    return assemble_output(res.results)


# revision 44
# speedup vs baseline: 1.1700x; 1.0172x over previous
"""Trainium2 Bass kernel for nn_Attention_47562467836169 (Bahdanau-style attention).

Reference math (S=4096, B=128, H=64):
    q = dec @ Wq_w.T + Wq_b                      # [B,1,H]
    k = enc @ Wk_w.T + Wk_b                      # [B,S,H]
    score = tanh(q + k) @ W_w.T + W_b            # [B,S,1]
    attn = softmax(score, axis=S)
    context = attn @ enc                         # [B,1,H]

Design (pair-major streaming; ~60us/exec vs 207us for the PE-transpose
baseline, measured via NTFF on 8 axon trn2 cores):
  * Pure data parallel over B: 16 batches/core as 8 PAIRS of 2 batches
    (2 x 64 h-channels = 128 partitions). The loop is PAIR-major: one
    pair's full 4096 s-rows per iteration, so the per-pair bias is
    CONSTANT per ACT instruction and bias+tanh+PSUM-eviction fuse into
    single [128,1024] ACT ops (the 352-cycle ACT overhead amortized; no
    separate DVE bias pass; ACT is the pacing engine at ~37us busy).
  * The encoder tensor is shipped in TWO host-prepared fp8-e4m3 layouts:
    h-partitioned (k-projection) and s-partitioned (context matmul) —
    zero on-device transposes, 8.5MB DMA/core. fp8 on both paths plus
    fp8 tanh output measures 1.15e-2 max rel err vs the f64 reference
    (gate 2e-2); bf16 encS would give 4.5e-3 at +4.25MB DMA.
  * The two streams ride different DMA paths so the 16 SDMA engines
    round-robin between queues (fills per-descriptor turnaround gaps):
    encT via sync HWDGE, encS via gpsimd SWDGE, both single_packet.
    Late-needed weights (v2/ones/ident) also go via gpsimd so the first
    encT stream starts ~2us earlier.
  * Matmul structure (FWL on all 128-col stationaries):
      - k:     stationary blockdiag(WkT,WkT) bf16, moving encT fp8
               [128,512] -> z in PSUM f32
      - v-dot: stationary tanh-chunk fp8 [128s,128] (FWL), moving
               v2 [128,2] -> scores come out s-PARTITIONED
      - ctx:   stationary enc-chunk fp8 [128s,128h2] (FWL), moving
               e-slice [128s,2], ONE PSUM accumulation group all kernel
      - den:   ones^T @ e per pair, DVE-reduced into den16 incrementally
  * softmax needs no max-subtraction (|score| <= sum|W_w| ~ 8); W_b is
    softmax-invariant and dropped; normalization happens once in the
    tail (transpose-via-PE + DVE reciprocal + per-partition scale).
  * exp(p) / den(p) / ctx(p) are deferred until after pair p+1's tanh is
    enqueued so ACT never stalls on pair p's v-dots (ACT stream runs
    ~92% packed).
  * PSUM: 6 banks z (3 x [128,1024] f32 waves -> PE runs 3 waves ahead,
    HAM stays warm), 1 bank scores+den (one-shot writes, 4-slot
    rotation), 1 bank ctx+tail scratch (also hosts the bias2 preamble).
  * Startup: pair-0's two streams are the FIRST instructions on their DMA
    queues (the tiny weights queue behind them, preamble runs off the cd
    bank, bias copy on the DVE) — worth ~10us of ramp vs singles-first.
"""

import numpy as np

S, B, H = 4096, 128, 64
NCORES = 8
BC = B // NCORES          # batches per core = 16
PAIRS = BC // 2           # 8
NC_ = S // 128            # 128-row chunks per pair = 32
WAVE = 1024               # s rows per z wave
NWAVE = S // WAVE         # 4

_CACHE = {}


def _build_nc(nblk=None, reps=1):
    import concourse.bacc as bacc
    import concourse.tile as tile
    from concourse import mybir

    f32 = mybir.dt.float32
    bf16 = mybir.dt.bfloat16
    fp8 = mybir.dt.float8e4
    Act = mybir.ActivationFunctionType

    nc = bacc.Bacc(None, target_bir_lowering=False)
    encT_d = nc.dram_tensor("enct", [PAIRS, 128, S], fp8, kind="ExternalInput")
    encS_d = nc.dram_tensor("encs", [PAIRS, 128, NC_, 128], fp8, kind="ExternalInput")
    decT_d = nc.dram_tensor("dect", [H + 2, BC], f32, kind="ExternalInput")
    wqT_d = nc.dram_tensor("wqt", [H + 2, H], f32, kind="ExternalInput")
    wk2_d = nc.dram_tensor("wk2", [128, 128], bf16, kind="ExternalInput")
    v2_d = nc.dram_tensor("v2", [128, 2], bf16, kind="ExternalInput")
    id_d = nc.dram_tensor("ident", [128, 128], f32, kind="ExternalInput")
    ones_d = nc.dram_tensor("ones1", [128, 1], bf16, kind="ExternalInput")
    out_d = nc.dram_tensor("outp", [BC, 128], f32, kind="ExternalOutput")

    with tile.TileContext(nc) as tc:
        with tc.tile_pool(name="singles", bufs=1) as singles:
            wk2_sb = singles.tile([128, 128], bf16)
            v2_sb = singles.tile([128, 2], bf16)
            id_sb = singles.tile([128, 128], f32)
            decT_sb = singles.tile([H + 2, BC], f32)
            wqT_sb = singles.tile([H + 2, H], f32)
            ones_sb = singles.tile([128, 1], bf16)
            bias2_sb = singles.tile([128, PAIRS], f32)
            one32_sb = singles.tile([1, 1], f32)
            warm_sb = singles.tile([1, 2], f32)
            den16_sb = singles.tile([1, BC], f32)
            rden_sb = singles.tile([BC, 1], f32)
            ctx_sb = singles.tile([128, BC], f32)
            outp_sb = singles.tile([BC, 128], f32)
            nc.vector.memset(one32_sb[:], 1.0)
            nc.vector.memset(warm_sb[:], 0.0)

            # ---- main streaming loop (pair-major)
            with (
                tc.tile_pool(name="etT", bufs=4) as etTp,
                tc.tile_pool(name="etS", bufs=4) as etSp,
                tc.tile_pool(name="thp", bufs=3) as thp,
                tc.tile_pool(name="epool", bufs=3) as epool,
                tc.tile_pool(name="psZ", bufs=3, space="PSUM") as psZ,
                tc.tile_pool(name="psS", bufs=1, space="PSUM") as psS,
                tc.tile_pool(name="psC", bufs=1, space="PSUM") as psC,
            ):
                # one bank, one-shot writes only: scores(p) at 64*(p%4),
                # den(p) at 256+64*(p%4); den is DVE-reduced into den16
                # before its slot is reused 4 pairs later.
                sc_ps = psS.tile([128, 512], f32, tag="sc")
                # one bank: cols 0:16 ctx, 80:81 rden^T, 128:256 ctx^T
                # (128:136 also hosts the one-shot bias2 preamble early on)
                cd_ps = psC.tile([128, 256], f32, tag="cd")

                # pair-0 streams issue FIRST on both queues so the k-matmul
                # pipeline is fed as early as possible; the tiny weights
                # queue up right behind them.
                pre0_etT = etTp.tile([128, S], fp8, tag="etT")
                nc.sync.dma_start(pre0_etT[:], encT_d[0], single_packet=True)
                pre0_etS = etSp.tile([128, NC_, 128], fp8, tag="etS")
                nc.gpsimd.dma_start(pre0_etS[:], encS_d[0], single_packet=True)
                nc.sync.dma_start(wk2_sb[:], wk2_d[:])
                nc.sync.dma_start(decT_sb[:], decT_d[:])
                nc.sync.dma_start(wqT_sb[:], wqT_d[:])
                nc.gpsimd.dma_start(v2_sb[:], v2_d[:])
                nc.gpsimd.dma_start(ones_sb[:], ones_d[:])
                nc.gpsimd.dma_start(id_sb[:], id_d[:])

                # Pull the exp_and_others ACT table (tanh+exp) off the
                # critical path, overlapped with the first pair's DMA.
                nc.scalar.activation(warm_sb[:, 1:2], warm_sb[:, 0:1], Act.Tanh)

                # preamble: bias2[64j+h', p] = (dec[2p+j] @ WqT + Wq_b + Wk_b)[h']
                nc.tensor.matmul(cd_ps[0:H, 128:136], wqT_sb[:],
                                 decT_sb[:, 0:PAIRS], start=True, stop=True,
                                 skip_group_check=True)
                nc.tensor.matmul(cd_ps[H:2 * H, 128:136], wqT_sb[:],
                                 decT_sb[:, PAIRS:BC], start=True, stop=True,
                                 tile_position=(0, 64), skip_group_check=True)
                nc.vector.tensor_copy(bias2_sb[:], cd_ps[:, 128:136])

                for rep in range(reps):
                    # softmax+ctx for pair p are deferred until after pair
                    # p+1's tanh is enqueued, so the ACT engine never stalls
                    # waiting for pair p's v-dots.
                    def soft_ctx(p, etS):
                        c0 = 64 * (p % 4)
                        e_sb = epool.tile([128, 64], bf16, tag="e")
                        nc.scalar.activation(e_sb[:], sc_ps[:, c0:c0 + 64], Act.Exp)
                        dc = 256 + 64 * (p % 4)
                        nc.tensor.matmul(sc_ps[0:1, dc:dc + 64], ones_sb[:],
                                         e_sb[:], start=True, stop=True,
                                         skip_group_check=True)
                        den_r = sc_ps[0:1, dc:dc + 64].rearrange(
                            "o (c j) -> o j c", j=2)
                        nc.vector.tensor_reduce(
                            den16_sb[0:1, 2 * p:2 * p + 2], den_r,
                            axis=mybir.AxisListType.X, op=mybir.AluOpType.add)
                        for c in range(NC_):
                            nc.tensor.matmul(
                                cd_ps[:, 2 * p:2 * p + 2],
                                etS[:, c, :],
                                e_sb[:, 2 * c:2 * c + 2],
                                start=(p == 0 and c == 0),
                                stop=(p == PAIRS - 1 and c == NC_ - 1),
                                skip_group_check=True)

                    prev = None
                    for p in range(PAIRS):
                        if rep == 0 and p == 0:
                            etT, etS = pre0_etT, pre0_etS
                        else:
                            etT = etTp.tile([128, S], fp8, tag="etT")
                            nc.sync.dma_start(etT[:], encT_d[p], single_packet=True)
                            etS = etSp.tile([128, NC_, 128], fp8, tag="etS")
                            nc.gpsimd.dma_start(etS[:], encS_d[p], single_packet=True)
                        th = thp.tile([128, S], fp8, tag="th")
                        for w in range(NWAVE):
                            zw = psZ.tile([128, WAVE], f32, tag="z")
                            for g in range(2):
                                o = WAVE * w + 512 * g
                                nc.tensor.matmul(zw[:, 512 * g:512 * (g + 1)],
                                                 wk2_sb[:], etT[:, o:o + 512],
                                                 start=True, stop=True)
                            nc.scalar.activation(th[:, WAVE * w:WAVE * (w + 1)],
                                                 zw[:], Act.Tanh,
                                                 bias=bias2_sb[:, p:p + 1])
                        c0 = 64 * (p % 4)
                        for c in range(NC_):
                            nc.tensor.matmul(
                                sc_ps[:, c0 + 2 * c:c0 + 2 * c + 2],
                                th[:, 128 * c:128 * (c + 1)],
                                v2_sb[:], start=True, stop=True)
                        if prev is not None:
                            soft_ctx(*prev)
                        prev = (p, etS)
                    soft_ctx(*prev)

                    # ---- tail: normalize and emit
                    nc.tensor.matmul(cd_ps[0:BC, 80:81], den16_sb[0:1, :],
                                     one32_sb[0:1, 0:1], is_transpose=True,
                                     start=True, stop=True, skip_group_check=True)
                    nc.vector.reciprocal(rden_sb[:], cd_ps[0:BC, 80:81])
                    nc.vector.tensor_copy(ctx_sb[:], cd_ps[:, 0:BC])
                    nc.tensor.matmul(cd_ps[0:BC, 128:256], ctx_sb[:], id_sb[:],
                                     is_transpose=True,
                                     start=True, stop=True, skip_group_check=True)
                    nc.vector.tensor_scalar_mul(outp_sb[:], cd_ps[0:BC, 128:256],
                                                rden_sb[:, 0:1])
                    nc.sync.dma_start(out_d[:], outp_sb[:])
    nc.compile()
    return nc


def get_nc(nblk=None, reps=1):
    key = (nblk, reps)
    if key not in _CACHE:
        _CACHE[key] = _build_nc(nblk, reps)
    return _CACHE[key]


def host_prep(enc, dec, wq_w, wq_b, wk_w, wk_b, w_w, nblk=None):
    """Build the 8 per-core input maps. enc [S,B,H] f32, dec [B,H]."""
    import ml_dtypes

    bf16 = ml_dtypes.bfloat16
    fp8 = ml_dtypes.float8_e4m3
    wk2 = np.zeros((128, 128), np.float32)
    wk2[0:H, 0:H] = wk_w.T
    wk2[H:2 * H, H:2 * H] = wk_w.T
    wk2 = wk2.astype(bf16)
    v2 = np.zeros((128, 2), np.float32)
    v2[0:H, 0] = w_w[0]
    v2[H:2 * H, 1] = w_w[0]
    v2 = v2.astype(bf16)
    ident = np.eye(128, dtype=np.float32)
    wqT = np.zeros((H + 2, H), np.float32)
    wqT[0:H] = wq_w.T
    wqT[H] = wq_b
    wqT[H + 1] = wk_b
    ones1 = np.ones((128, 1), np.float32).astype(bf16)
    in_maps = []
    for c in range(NCORES):
        e = enc[:, BC * c:BC * (c + 1), :]            # [S, 16, 64] f32
        # h-partitioned: encT[p, 64j+h, s] = e[s, 2p+j, h]
        et = e.reshape(S, PAIRS, 2, H)                 # [s, p, j, h]
        encT = np.ascontiguousarray(
            et.transpose(1, 2, 3, 0).reshape(PAIRS, 128, S)
        ).astype(fp8)
        # s-partitioned: encS[p, i, c, 64j+h] = e[128c+i, 2p+j, h]
        es = e.reshape(NC_, 128, PAIRS, 2 * H)         # [c, i, p, jh]
        encS = np.ascontiguousarray(
            es.transpose(2, 1, 0, 3).reshape(PAIRS, 128, NC_, 128)
        ).astype(fp8)
        d = dec[BC * c:BC * (c + 1)]                  # [16, 64]
        decT = np.ones((H + 2, BC), np.float32)
        decT[0:H, 0:PAIRS] = d[0::2].T
        decT[0:H, PAIRS:BC] = d[1::2].T
        in_maps.append({
            "enct": encT, "encs": encS,
            "dect": decT, "wqt": wqT, "wk2": wk2, "v2": v2, "ident": ident,
            "ones1": ones1,
        })
    return in_maps


def assemble_output(results):
    """results: list of 8 dicts with 'outp' [16,128] -> full [1,B,H]."""
    out = np.zeros((1, B, H), np.float32)
    for c in range(NCORES):
        o = results[c]["outp"]                        # [16, 128]
        for r in range(BC):
            j = r % 2
            out[0, BC * c + r, :] = o[r, H * j:H * (j + 1)]
    return out


def kernel(encoder_outputs, decoder_hidden, Wq_w, Wq_b, Wk_w, Wk_b, W_w, W_b,
           **kwargs):
    from concourse.bass_utils import run_bass_kernel_spmd

    enc = np.asarray(encoder_outputs, np.float32)
    dec = np.asarray(decoder_hidden, np.float32)[0]
    in_maps = host_prep(enc, dec,
                        np.asarray(Wq_w, np.float32), np.asarray(Wq_b, np.float32),
                        np.asarray(Wk_w, np.float32), np.asarray(Wk_b, np.float32),
                        np.asarray(W_w, np.float32))
    nc = get_nc()
    res = run_bass_kernel_spmd(nc, in_maps, core_ids=list(range(NCORES)))
    return assemble_output(res.results)
